# revision 1
# baseline (speedup 1.0000x reference)
"""Trainium2 Bass kernel for nn_ExtractionLayer.

metric[b,v,f] = sum_p amp[b,f,p] * exp(-c*(vol[v]*filt[f] - q[b,p])^2)
  amp = softmax_p(logits[b,f,p]),  c = 0.5/(sigma+0.001)^2

Sharding: data-parallel over batch B=32 -> 4 b's per core on 8 cores.

Per-core algorithm (2 "sets", each set = 2 b's = 128 (b,p) partition pairs):
  PE pass 1 : S'[(b,p),(f,v)] = x^2 - 2qx via a K=9 bf16 matmul.
              bf16 is 4x faster than fp32 on the PE; fp32-level accuracy
              comes from 3-way hi/mid/lo bf16 splits of x^2, x and -2q
              (9 rows = 3 a-rows + {b_h*3, b_m*2, b_l*1} cross terms).
              Even/odd 512-col chunks use PE row-groups 0/32 and run
              concurrently. The q^2 term folds into the ACT bias.
  ACT pass  : E = exp(-c*S' - c*q^2)  PSUM->SBUF fp16, FD=1536 groups,
              double-buffered PSUM -- this ~55us exp pass is the floor.
  PE pass 2 : per (f, v-half): lhsT = E-slice (128,128) stationary,
              rhs = block-diag softmax weight pair (128,2) moving ->
              psum out (128 v, 2 b) -- partition-dense output.
  drain     : DVE copy psum->SBUF, DMA -> out[s,v,f,b']; host -> [b,v,f]

X rows are built on device (DVE splits of fil*vol outer product),
bounced through DRAM, and streamed into 4 column-piece tiles so PE-1
starts as soon as the first piece lands.
"""

import sys

for _p in ("/opt/trn_rl_repo", "/root/.axon_site/_ro/trn_rl_repo"):
    if _p not in sys.path:
        sys.path.append(_p)

import numpy as np

B, V, F, P = 32, 256, 128, 64
NCORES = 8
B_LOC = B // NCORES          # 4 batches per core
NSETS = B_LOC // 2           # 2 sets of (2 b's x 64 p) = 128 partitions
NVF = V * F                  # 32768 (f-major: i = f*V + v)
GROUP = 1536                 # ACT free dim (3 PSUM banks); last group ragged
# per-set group list: (start_col, n_cols), n_cols multiple of V
GROUPS = []
_c0 = 0
while _c0 < NVF:
    GROUPS.append((_c0, min(GROUP, NVF - _c0)))
    _c0 += GROUP

_cache: dict = {}


def _build(minus_c):
    import concourse.tile as tile
    from concourse import bacc, mybir

    fp32 = mybir.dt.float32
    fp16 = mybir.dt.float16
    bf16 = mybir.dt.bfloat16
    AF = mybir.ActivationFunctionType
    OP = mybir.AluOpType
    import concourse.bass as bass

    nc = bacc.Bacc("TRN2", target_bir_lowering=False, debug=False,
                   num_devices=NCORES)

    d_q = nc.dram_tensor("q", [B_LOC * P], fp32, kind="ExternalInput")
    d_lf = nc.dram_tensor("lf", [F, B_LOC, P], fp32, kind="ExternalInput")
    d_lt = nc.dram_tensor("lt", [B_LOC * P, F], fp32, kind="ExternalInput")
    d_vol = nc.dram_tensor("vol", [128, V], fp32, kind="ExternalInput")
    d_fil = nc.dram_tensor("fil", [F], fp32, kind="ExternalInput")
    d_sig = nc.dram_tensor("sig", [1], fp32, kind="ExternalInput")
    # out[s, v, f, b'] -> contiguous DMA per (set, v-half); host interleaves
    d_out = nc.dram_tensor("out", [NSETS, V, F, 2], fp32, kind="ExternalOutput")
    d_zb = nc.dram_tensor("zb", [B_LOC * F], fp32)  # Zinv bounce, [b][f]
    # split bounces: piece p depends only on its f-quarter q=p
    d_xb_b = [nc.dram_tensor(f"xbb{q}", [32, 3 * V], bf16) for q in range(4)]
    d_xb_a = [nc.dram_tensor(f"xba{q}", [32, 3 * V], bf16) for q in range(4)]

    with tile.TileContext(nc) as tc:
        with (
            tc.tile_pool(name="const", bufs=1) as cp,
            tc.tile_pool(name="ering", bufs=4) as ep,
            tc.tile_pool(name="psS", bufs=2, space=bass.MemorySpace.PSUM) as psS,
            tc.tile_pool(name="psO", bufs=2, space=bass.MemorySpace.PSUM) as psO,
        ):
            # round-robin DMA issue over engines that sit mostly idle
            dmaeng = [nc.sync, nc.gpsimd]
            dmactr = [0]

            def dma(dst, src):
                e = dmaeng[dmactr[0] % len(dmaeng)]
                dmactr[0] += 1
                e.dma_start(dst, src)

            # setup-only rotation may also use the ACT engine's DMA port
            dmaeng3 = [nc.sync, nc.gpsimd, nc.scalar]

            def dma3(dst, src):
                e = dmaeng3[dmactr[0] % len(dmaeng3)]
                dmactr[0] += 1
                e.dma_start(dst, src)

            # ---- queue plan ----
            # sync   : q/fil/vol loads, packed->DRAM bounce, X bands 0-1
            # scalar : softmax exps, Wq DMAs, X bands 2-3, then main exps
            # gpsimd : lf/lt loads, zb/zr chain, steady-state output DMAs
            # vector : x/splits, Z/Zinv, srow, Wamp, drains
            volr = cp.tile([128, V], fp32, tag="volr")
            nc.sync.dma_start(volr[:, :], d_vol.ap())
            filc = cp.tile([128, 1], fp32, tag="filc")
            nc.sync.dma_start(filc[:, :], d_fil.ap().rearrange("(f o) -> f o", o=1))
            q_row = cp.tile([1, B_LOC * P], fp32, tag="q_row")
            nc.sync.dma_start(q_row[:, :], d_q.ap())
            qcol = cp.tile([128, NSETS], fp32, tag="qcol")
            for s in range(NSETS):
                nc.gpsimd.dma_start(
                    qcol[:, s:s + 1],
                    d_q.ap().rearrange("(k o) -> k o", o=1)[s * 128:(s + 1) * 128, :])
            lf_sb = cp.tile([128, B_LOC, P], fp32, tag="lf_sb")
            nc.gpsimd.dma_start(lf_sb[:, :, :], d_lf.ap())
            lt_sb = []
            for s in range(NSETS):
                t = cp.tile([128, F], fp32, tag=f"lt{s}", name=f"lt{s}")
                nc.gpsimd.dma_start(t[:, :], d_lt.ap()[s * 128:(s + 1) * 128, :])
                lt_sb.append(t)

            # softmax exps early on the ACT queue
            el_f = cp.tile([128, B_LOC, P], fp32, tag="el_f")
            nc.scalar.activation(el_f[:, :, :], lf_sb[:, :, :], AF.Exp)
            elt = []
            for s in range(NSETS):
                e = cp.tile([128, F], fp32, tag=f"elt{s}", name=f"elt{s}")
                nc.scalar.activation(e[:, :], lt_sb[s][:, :], AF.Exp)
                elt.append(e)

            # ---- x = fil*vol and bf16 splits (DVE), packed col-blocks ----
            # packed[:, blk*V:(blk+1)*V] blocks: a_h a_m a_l b_h b_m b_l 1
            x_ft = cp.tile([128, V], fp32, tag="x_ft")
            nc.vector.tensor_scalar(x_ft[:, :], volr[:, :], filc[:, 0:1], None,
                                    op0=OP.mult)
            xsq_ft = cp.tile([128, V], fp32, tag="xsq_ft")
            nc.vector.tensor_tensor(xsq_ft[:, :], x_ft[:, :], x_ft[:, :], OP.mult)
            packed = cp.tile([128, 6 * V], bf16, tag="packed")
            # blocks 0-2: b=x splits (bounced first), 3-5: a=x^2 splits

            def split3(val32, blk, eng):
                """bf16-split val32 into packed blocks blk, blk+1, blk+2."""
                r1 = cp.tile([128, V], fp32, tag=f"r1_{blk}", name=f"r1_{blk}")
                r2 = cp.tile([128, V], fp32, tag=f"r2_{blk}", name=f"r2_{blk}")
                h = packed[:, blk * V:(blk + 1) * V]
                m = packed[:, (blk + 1) * V:(blk + 2) * V]
                l = packed[:, (blk + 2) * V:(blk + 3) * V]
                eng.tensor_copy(h, val32[:, :])
                eng.tensor_tensor(r1[:, :], val32[:, :], h, OP.subtract)
                eng.tensor_copy(m, r1[:, :])
                eng.tensor_tensor(r2[:, :], r1[:, :], m, OP.subtract)
                eng.tensor_copy(l, r2[:, :])

            split3(x_ft, 0, nc.vector)
            for q in range(4):
                bap = bass.AP(tensor=d_xb_b[q], offset=0,
                              ap=[[V, 32], [32 * V, 3], [1, V]])
                nc.scalar.dma_start(bap, packed[32 * q:32 * (q + 1), 0:3 * V])
            split3(xsq_ft, 3, nc.vector)
            for q in range(4):
                aap = bass.AP(tensor=d_xb_a[q], offset=0,
                              ap=[[V, 32], [32 * V, 3], [1, V]])
                nc.scalar.dma_start(aap, packed[32 * q:32 * (q + 1), 3 * V:6 * V])

            # exp bias per set: -c * q^2  (folds the q^2 matmul rows away)
            ebias = cp.tile([128, NSETS], fp32, tag="ebias")
            nc.vector.tensor_tensor(ebias[:, :], qcol[:, :], qcol[:, :], OP.mult)
            nc.vector.tensor_scalar_mul(ebias[:, :], ebias[:, :], float(minus_c))
            # ---- X: two row-group bands (PE rows 0-8 / 32-40) so even/odd
            # chunks run as concurrent matmuls in different 32-row groups.
            # 4 column-piece tiles stream in while PE consumes them.
            # Piece tile rows 0-8 = band 0 (even chunks), 32-40 = band 1.
            # uneven pieces: small first pieces -> earliest PE start
            PJ = [(0, 4), (4, 4), (8, 8), (16, 8), (24, 8)]  # (j0, nj)
            PB = [0, 2048, 4096, 8192, 12288, 16384]          # col bounds
            NP = 8  # j-dim stride unit in source AP (8 j's per f-quarter)
            Xp = [cp.tile([41, nj * 512], bf16, tag=f"Xp{p}", name=f"Xp{p}")
                  for p, (j0, nj) in enumerate(PJ)]
            # (dst_row0, n_rows, src_block, src_row_stride_in_blocks, bounce)
            rowgrps = [(3, 3, 0, 0, d_xb_b), (6, 2, 1, 0, d_xb_b),
                       (8, 1, 2, 0, d_xb_b), (0, 3, 0, 1, d_xb_a)]
            xeng = [nc.sync, nc.gpsimd]
            xi = 0

            def load_piece(p):
                # piece p covers within-band chunks j in [j0, j0+nj); band b
                # chunk j holds f = 4j + 2b + fi (f-major, 512-col chunks);
                # bounce layout [blk][f][v] makes each chunk a 512-run
                nonlocal xi
                j0, nj = PJ[p]
                q = j0 // 8                           # source f-quarter
                for band in range(2):
                    for (r0, nr, blk, rstr, dxb) in rowgrps:
                        srcap = bass.AP(
                            tensor=dxb[q],
                            offset=blk * 32 * V + (2 * band) * V
                                   + (j0 % 8) * 4 * V,
                            ap=[[rstr * 32 * V, nr], [4 * V, nj],
                                [1, 2 * V]],
                        )
                        dstap = Xp[p][32 * band + r0:32 * band + r0 + nr, :]
                        dstap = dstap.rearrange("r (j w) -> r j w", j=nj)
                        xeng[xi % 2].dma_start(dstap, srcap)
                        xi += 1

            load_piece(0)
            load_piece(1)
            load_piece(2)

            # ---- W_q rows (DVE srow chains), DMAs on scalar ----
            Wq = []
            wt32 = cp.tile([1, 128], fp32, tag="wt32")
            res1 = cp.tile([1, 128], fp32, tag="res1")
            res2 = cp.tile([1, 128], fp32, tag="res2")

            def split3_row(val32, srow, cols):
                """bf16-split val32 (1,128) into 128-col slices of srow.
                Runs on gpsimd: tiny ops, keeps DVE free for the X splits."""
                cur = val32
                for i, cidx in enumerate(cols):
                    hb = srow[0:1, cidx * 128:(cidx + 1) * 128]
                    nc.vector.tensor_copy(hb, cur[:, :])
                    if i < len(cols) - 1:
                        dst = res1 if cur is not res1 else res2
                        nc.vector.tensor_tensor(dst[:, :], cur[:, :], hb,
                                                OP.subtract)
                        cur = dst

            for s in range(NSETS):
                srow = cp.tile([1, 9 * 128], bf16, tag=f"srow{s}",
                               name=f"srow{s}")
                w = cp.tile([41, 128], bf16, tag=f"Wq{s}", name=f"Wq{s}")
                qs = q_row[0:1, s * 128:(s + 1) * 128]
                nc.vector.memset(srow[0:1, 0:3 * 128], 1.0)
                nc.vector.tensor_scalar_mul(wt32[:, :], qs, -2.0)
                split3_row(wt32, srow, [3, 4, 5])   # w_h w_m w_l
                nc.vector.tensor_copy(srow[0:1, 6 * 128:7 * 128],
                                      srow[0:1, 3 * 128:4 * 128])  # w_h
                nc.vector.tensor_copy(srow[0:1, 7 * 128:8 * 128],
                                      srow[0:1, 4 * 128:5 * 128])  # w_m
                nc.vector.tensor_copy(srow[0:1, 8 * 128:9 * 128],
                                      srow[0:1, 3 * 128:4 * 128])  # w_h
                nc.scalar.dma_start(w[0:9, :], srow[0:1, :])
                nc.scalar.dma_start(w[32:41, :], srow[0:1, :])
                Wq.append(w)

            # ---- softmax denominators (DVE) + zb/zr chain (gpsimd) ----
            Z = cp.tile([128, B_LOC], fp32, tag="Z")
            nc.vector.tensor_reduce(Z[:, :], el_f[:, :, :], mybir.AxisListType.X,
                                    OP.add)
            Zinv = cp.tile([128, B_LOC], fp32, tag="Zinv")
            nc.vector.reciprocal(Zinv[:, :], Z[:, :])
            nc.gpsimd.dma_start(
                bass.AP(tensor=d_zb, offset=0, ap=[[1, 128], [128, B_LOC]]),
                Zinv[:, :])
            zrs = []
            for s in range(NSETS):
                zr = cp.tile([128, F], fp32, tag=f"zr{s}", name=f"zr{s}")
                for h in range(2):
                    nc.gpsimd.dma_start(
                        zr[h * 64:(h + 1) * 64, :],
                        bass.AP(tensor=d_zb, offset=(2 * s + h) * F,
                                ap=[[0, 64], [1, F]]))
                zrs.append(zr)

            load_piece(3)
            load_piece(4)

            # ---- W_amp: block-diag fp16 softmax weights (DVE) ----
            # W_amp[k=(b,p), 2f+h] = amp[b,f,p] for k//64==h else 0
            Wamp = []
            for s in range(NSETS):
                w = cp.tile([128, 2 * F], fp16, tag=f"Wamp{s}", name=f"Wamp{s}")
                nc.vector.memset(w[:, :], 0.0)
                for h in range(2):
                    nc.vector.tensor_tensor(
                        w[h * 64:(h + 1) * 64, h:2 * F:2],
                        elt[s][h * 64:(h + 1) * 64, :],
                        zrs[s][h * 64:(h + 1) * 64, :],
                        OP.mult,
                    )
                Wamp.append(w)

            # ---- main pipeline ----
            for s in range(NSETS):
                # psum out accumulator per set, cols [vh*256 + 2*f + b']
                sO = psO.tile([128, 2 * 2 * F], fp32, tag="O", name="sO")
                for (g0, gc) in GROUPS:
                    sS = psS.tile([128, GROUP], fp32, tag="S", name="sS")
                    for ci in range(gc // 512):
                        c = (g0 + ci * 512) // 512        # global chunk
                        band, bloc = c % 2, (c // 2) * 512
                        p = next(k for k in range(5)
                                 if PB[k] <= bloc < PB[k + 1])
                        loc = bloc - PB[p]
                        nc.tensor.matmul(
                            sS[:, ci * 512:(ci + 1) * 512],
                            Wq[s][32 * band:32 * band + 9, :],
                            Xp[p][32 * band:32 * band + 9, loc:loc + 512],
                            start=True, stop=True,
                            tile_position=(32 * band, 0),
                        )
                    E = ep.tile([128, GROUP], fp16, tag="E", name="E")
                    nc.scalar.activation(E[:, 0:gc], sS[:, 0:gc], AF.Exp,
                                         scale=float(minus_c),
                                         bias=ebias[:, s:s + 1])
                    for fr in range(gc // V):
                        f = g0 // V + fr                       # global f
                        for vh in range(2):
                            base = vh * 2 * F
                            nc.tensor.matmul(
                                sO[:, base + 2 * f:base + 2 * f + 2],
                                E[:, fr * V + vh * 128:fr * V + vh * 128 + 128],
                                Wamp[s][:, 2 * f:2 * f + 2],
                                start=True, stop=True,
                            )
                # drain psum -> sbuf -> DRAM out[s, v, f, b'] (contiguous)
                for vh in range(2):
                    ob = cp.tile([128, 2 * F], fp32, tag=f"ob{vh}",
                                 name=f"ob{vh}")
                    base = vh * 2 * F
                    nc.vector.tensor_copy(ob[:, :], sO[:, base:base + 2 * F])
                    nc.gpsimd.dma_start(
                        d_out.ap()[s:s + 1, vh * 128:(vh + 1) * 128, :, :],
                        ob[:, :],
                    )

    nc.compile()
    return nc


def _get_nc(minus_c):
    key = float(minus_c)
    if key not in _cache:
        _cache[key] = _build(key)
    return _cache[key]


def kernel(q2_obs_scaled, amplitude_logits, volumes, filters, sigma,
           _trace=False, _tmpdir=None):
    from concourse.bass_utils import run_bass_kernel_spmd

    minus_c = -0.5 / (float(np.asarray(sigma).reshape(())) + 0.001) ** 2
    nc = _get_nc(minus_c)

    q = np.ascontiguousarray(np.asarray(q2_obs_scaled, dtype=np.float32))
    lg = np.asarray(amplitude_logits, dtype=np.float32).reshape(B, F, P)
    vol = np.ascontiguousarray(np.asarray(volumes, dtype=np.float32).reshape(V))
    fil = np.ascontiguousarray(np.asarray(filters, dtype=np.float32).reshape(F))
    sig = np.asarray(sigma, dtype=np.float32).reshape(1)

    in_maps = []
    for i in range(NCORES):
        bsl = slice(i * B_LOC, (i + 1) * B_LOC)
        lgc = lg[bsl]                                    # (B_LOC, F, P)
        in_maps.append({
            "q": np.ascontiguousarray(q[bsl].reshape(B_LOC * P)),
            "lf": np.ascontiguousarray(lgc.transpose(1, 0, 2)),   # (F,B_LOC,P)
            "lt": np.ascontiguousarray(
                lgc.transpose(0, 2, 1).reshape(B_LOC * P, F)),    # ((b,p),F)
            "vol": np.broadcast_to(vol, (128, V)).copy(),
            "fil": fil,
            "sig": sig,
        })

    kw = {}
    if _trace:
        kw = {"trace": True, "tmpdir": _tmpdir}
    res = run_bass_kernel_spmd(nc, in_maps, core_ids=list(range(NCORES)), **kw)

    out = np.empty((B, V, F), dtype=np.float32)
    for i in range(NCORES):
        oc = res.results[i]["out"]                       # (NSETS, V, F, 2)
        for s in range(NSETS):
            for h in range(2):
                out[i * B_LOC + 2 * s + h] = oc[s, :, :, h]
    if _trace:
        return out, res
    return out



# revision 4
# speedup vs baseline: 1.2443x; 1.2443x over previous
"""Trainium2 Bass kernel for nn_ExtractionLayer (v2, transposed layout).

metric[b,v,f] = sum_p amp[b,f,p] * exp(-c*(vol[v]*filt[f] - q[b,p])^2)
  amp = softmax_p(logits[b,f,p]),  c = 0.5/(sigma+0.001)^2

Sharding: data-parallel over batch B=32 -> 4 b's per core on 8 cores.

Per-core algorithm (one PE pass, one ACT pass, one DVE pass):
  Layout: 256 chunks c=(f,vh); PSUM rows = 128 v's of the chunk,
  free cols = (b,p) = 4*64.
  PE    : S[v',(b,p)] = x^2 - 2qx + q^2 - lnamp/c via a K=12 bf16 matmul
          per chunk. x-side splits (x^2 h/m/l, x h/m/l) are the
          STATIONARY operand; q-side splits (w=-2q h/m/l) and
          phi = q^2 - lnamp/c h/m/l are the MOVING operand. The softmax
          amplitude is folded into the exponent (phi), so no second PE
          pass is needed. Even/odd chunks alternate PE row groups 0/32.
  ACT   : E = exp(-c*S) PSUM->SBUF fp16, 2048-col groups (8 chunks),
          double-buffered PSUM. amp is already inside the exponent.
  DVE   : R[v', 4*c+b] = sum_p E  (segmented 64-col tensor_reduce).
  drain : quarter-wise DMA of R -> out[128, 1024]; host reorders.

ALL small tensors (x splits, q splits, phi = q^2 - log softmax / c) are
precomputed on host in fp64 and shipped as two bf16 input tiles, so the
device does nothing but DMA-in, matmul, exp, reduce, DMA-out.
"""

import sys

for _p in ("/opt/trn_rl_repo", "/root/.axon_site/_ro/trn_rl_repo"):
    if _p not in sys.path:
        sys.path.append(_p)

import numpy as np
import ml_dtypes

BF16 = ml_dtypes.bfloat16

B, V, F, P = 32, 256, 128, 64
NCORES = 8
B_LOC = B // NCORES          # 4 batches per core
NCH = 2 * F                  # 256 chunks: c = f*2 + vh
CPG = 8                      # chunks per PSUM group
NGRP = NCH // CPG            # 32 groups
GCOLS = CPG * B_LOC * P      # 2048 cols per group
NK = 12                      # matmul contraction rows

_cache: dict = {}


def _build(minus_c):
    import concourse.tile as tile
    from concourse import bacc, mybir

    fp32 = mybir.dt.float32
    fp16 = mybir.dt.float16
    bf16 = mybir.dt.bfloat16
    AF = mybir.ActivationFunctionType
    OP = mybir.AluOpType
    import concourse.bass as bass

    nc = bacc.Bacc("TRN2", target_bir_lowering=False, debug=False,
                   num_devices=NCORES)

    # stationary x-side rows: [band*32 + r, j*128 + v']
    d_xst = nc.dram_tensor("xst", [44, F * 128], bf16, kind="ExternalInput")
    # moving q-side rows:     [band*32 + r, j*256 + (b*64+p)]
    d_wmv = nc.dram_tensor("wmv", [44, F * 256], bf16, kind="ExternalInput")
    # out[v', c*4 + b]
    d_out = nc.dram_tensor("out", [128, NCH * B_LOC], fp32,
                           kind="ExternalOutput")

    with tile.TileContext(nc) as tc:
        with (
            tc.tile_pool(name="const", bufs=1) as cp,
            tc.tile_pool(name="ering", bufs=3) as ep,
            tc.tile_pool(name="psS", bufs=2, space=bass.MemorySpace.PSUM) as psS,
        ):
            # preload the ACT exp table while input DMAs are in flight
            warm = cp.tile([1, 2], fp32, tag="warm")
            nc.vector.memset(warm[:, :], 0.0)
            nc.scalar.activation(warm[:, 0:1], warm[:, 1:2], AF.Exp)

            xst = cp.tile([44, F * 128], bf16, tag="xst")
            wmv = cp.tile([44, F * 256], bf16, tag="wmv")
            R = cp.tile([128, NCH * B_LOC], fp32, tag="R")

            # input pieces: small first (fast pipeline start), then coarse
            pieces = [1, 1, 2, 4, 4, 4, 4, 4, 4, 4]
            g0 = 0
            for ng in pieces:
                nc.sync.dma_start(xst[:, g0 * 512:(g0 + ng) * 512],
                                  d_xst.ap()[:, g0 * 512:(g0 + ng) * 512])
                nc.gpsimd.dma_start(wmv[:, g0 * 1024:(g0 + ng) * 1024],
                                    d_wmv.ap()[:, g0 * 1024:(g0 + ng) * 1024])
                g0 += ng

            for g in range(NGRP):
                sS = psS.tile([128, GCOLS], fp32, tag="S", name="sS")
                for cc in range(CPG):
                    c = g * CPG + cc
                    j, band = c >> 1, c & 1
                    r0 = 32 * band
                    # concurrent row-tile matmuls must not share a PSUM
                    # bank: band 0 -> banks 0-1, band 1 -> banks 2-3
                    pc = (cc & 1) * 1024 + (cc >> 1) * 256
                    nc.tensor.matmul(
                        sS[:, pc:pc + 256],
                        xst[r0:r0 + NK, j * 128:(j + 1) * 128],
                        wmv[r0:r0 + NK, j * 256:(j + 1) * 256],
                        start=True, stop=True,
                        tile_position=(r0, 0),
                    )
                E = ep.tile([128, GCOLS], fp16, tag="E", name="E")
                nc.scalar.activation(E[:, :], sS[:, :], AF.Exp,
                                     scale=float(minus_c))
                nc.vector.tensor_reduce(
                    R[:, g * 32:(g + 1) * 32],
                    E[:, :].rearrange("p (s x) -> p s x", x=P),
                    mybir.AxisListType.X, OP.add)
                if g % 8 == 7:
                    qtr = g // 8
                    nc.sync.dma_start(
                        d_out.ap()[:, qtr * 256:(qtr + 1) * 256],
                        R[:, qtr * 256:(qtr + 1) * 256])

    nc.compile()
    return nc


def _get_nc(minus_c):
    key = float(minus_c)
    if key not in _cache:
        _cache[key] = _build(key)
    return _cache[key]


def _split3(v):
    """3-way bf16 split of an fp64 array: h + m + l ~= v to ~24 bits."""
    h = v.astype(BF16)
    r = v - h.astype(np.float64)
    m = r.astype(BF16)
    r2 = r - m.astype(np.float64)
    l = r2.astype(BF16)
    return h, m, l


def kernel(q2_obs_scaled, amplitude_logits, volumes, filters, sigma,
           _trace=False, _tmpdir=None):
    from concourse.bass_utils import run_bass_kernel_spmd

    sig = float(np.asarray(sigma).reshape(()))
    minus_c = -0.5 / (sig + 0.001) ** 2
    c = -minus_c
    nc = _get_nc(minus_c)

    q = np.asarray(q2_obs_scaled, np.float64)                    # (B, P)
    lg = np.asarray(amplitude_logits, np.float64).reshape(B, F, P)
    vol = np.asarray(volumes, np.float64).reshape(V)
    fil = np.asarray(filters, np.float64).reshape(F)

    # log softmax over p
    mx = lg.max(axis=2, keepdims=True)
    lnamp = lg - (mx + np.log(np.exp(lg - mx).sum(axis=2, keepdims=True)))

    # ---- stationary x-side tile (shared by all cores) ----
    x2d = vol[:, None] * fil[None, :]                            # (V, F)
    xst = np.zeros((44, F * 128), dtype=BF16)
    for band in range(2):
        xT = np.ascontiguousarray(x2d[band * 128:(band + 1) * 128, :].T)
        x2h, x2m, x2l = _split3(xT * xT)
        xh, xm, xl = _split3(xT)
        ones = np.ones_like(xT, dtype=BF16)
        rows = [x2h, x2m, x2l, xh, xh, xh, xm, xm, xl, ones, ones, ones]
        for r, arr in enumerate(rows):
            xst[32 * band + r, :] = np.ascontiguousarray(
                arr, dtype=BF16).reshape(-1)

    # ---- per-core moving q-side tiles ----
    in_maps = []
    for i in range(NCORES):
        qc = q[B_LOC * i:B_LOC * (i + 1), :]                     # (4, P)
        ln = lnamp[B_LOC * i:B_LOC * (i + 1), :, :]              # (4, F, P)
        wh, wm_, wl = _split3(-2.0 * qc)
        ph, pm, pl = _split3(qc[:, None, :] ** 2 - ln / c)       # (4, F, P)

        wmv = np.zeros((44, F * 256), dtype=BF16)
        onesn = np.ones(B_LOC * P, dtype=BF16)

        def flat(a):
            return np.ascontiguousarray(a, dtype=BF16).reshape(-1)

        const_rows = [onesn, onesn, onesn, flat(wh), flat(wm_), flat(wl),
                      flat(wh), flat(wm_), flat(wh)]
        for r, arr in enumerate(const_rows):
            rowv = np.tile(arr, F)
            wmv[r, :] = rowv
            wmv[32 + r, :] = rowv
        for r, arr in enumerate([ph, pm, pl]):
            rowv = flat(arr.transpose(1, 0, 2))                  # (F, 4, P)
            wmv[9 + r, :] = rowv
            wmv[32 + 9 + r, :] = rowv
        in_maps.append({"xst": xst, "wmv": wmv})

    kw = {}
    if _trace:
        kw = {"trace": True, "tmpdir": _tmpdir}
    res = run_bass_kernel_spmd(nc, in_maps, core_ids=list(range(NCORES)), **kw)

    # chunk c = f*2+vh sits at R col base (c//8)*32 + (c%8&1)*16 + (c%8>>1)*4
    cs = np.arange(NCH)
    colbase = (cs // CPG) * 32 + ((cs % CPG) & 1) * 16 + ((cs % CPG) >> 1) * 4
    cols = colbase[:, None] + np.arange(B_LOC)[None, :]          # (NCH, 4)

    out = np.empty((B, V, F), dtype=np.float32)
    for i in range(NCORES):
        Rg = res.results[i]["out"][:, cols]                      # (128, NCH, 4)
        Rr = Rg.reshape(128, F, 2, B_LOC)                        # v',f,vh,b
        out[B_LOC * i:B_LOC * (i + 1)] = (
            Rr.transpose(3, 2, 0, 1).reshape(B_LOC, V, F))
    if _trace:
        return out, res
    return out


# revision 8
# speedup vs baseline: 1.4305x; 1.1497x over previous
"""Trainium2 Bass kernel for nn_ExtractionLayer (v2, transposed layout).

metric[b,v,f] = sum_p amp[b,f,p] * exp(-c*(vol[v]*filt[f] - q[b,p])^2)
  amp = softmax_p(logits[b,f,p]),  c = 0.5/(sigma+0.001)^2

Sharding: data-parallel over batch B=32 -> 4 b's per core on 8 cores.

Per-core algorithm (one PE pass, one ACT pass, one DVE pass):
  Layout: 256 chunks c=(f,vh); PSUM rows = 128 v's of the chunk,
  free cols = (b,p) = 4*64.
  PE    : S[v',(b,p)] = x^2 - 2qx + q^2 - lnamp/c via a K=12 bf16 matmul
          per chunk. x-side splits (x^2 h/m/l, x h/m/l) are the
          STATIONARY operand; q-side splits (w=-2q h/m/l) and
          phi = q^2 - lnamp/c h/m/l are the MOVING operand. The softmax
          amplitude is folded into the exponent (phi), so no second PE
          pass is needed. Even/odd chunks alternate PE row groups 0/32.
  ACT   : E = exp(-c*S) PSUM->SBUF fp16, 2048-col groups (8 chunks),
          double-buffered PSUM. amp is already inside the exponent.
  DVE   : R[v', 4*c+b] = sum_p E  (segmented 64-col tensor_reduce).
  drain : quarter-wise DMA of R -> out[128, 1024]; host reorders.

ALL small tensors (x splits, q splits, phi = q^2 - log softmax / c) are
precomputed on host in fp64 and shipped as two bf16 input tiles, so the
device does nothing but DMA-in, matmul, exp, reduce, DMA-out.
"""

import sys

for _p in ("/opt/trn_rl_repo", "/root/.axon_site/_ro/trn_rl_repo"):
    if _p not in sys.path:
        sys.path.append(_p)

import numpy as np
import ml_dtypes

BF16 = ml_dtypes.bfloat16

B, V, F, P = 32, 256, 128, 64
NCORES = 8
B_LOC = B // NCORES          # 4 batches per core
NCH = 2 * F                  # 256 chunks: c = f*2 + vh
CPG = 8                      # chunks per PSUM group
NGRP = NCH // CPG            # 32 groups
GCOLS = CPG * B_LOC * P      # 2048 cols per group
NK = 12                      # matmul contraction rows

_cache: dict = {}


def _build(minus_c):
    import concourse.tile as tile
    from concourse import bacc, mybir

    fp32 = mybir.dt.float32
    fp16 = mybir.dt.float16
    bf16 = mybir.dt.bfloat16
    AF = mybir.ActivationFunctionType
    OP = mybir.AluOpType
    import concourse.bass as bass

    nc = bacc.Bacc("TRN2", target_bir_lowering=False, debug=False,
                   num_devices=NCORES)

    # stationary x-side rows: [band*32 + r, j*128 + v']
    d_xst = nc.dram_tensor("xst", [44, F * 128], bf16, kind="ExternalInput")
    # moving q-side rows:     [band*32 + r, j*256 + (b*64+p)]
    d_wmv = nc.dram_tensor("wmv", [44, F * 256], bf16, kind="ExternalInput")
    # out[v', c*4 + b]
    d_out = nc.dram_tensor("out", [128, NCH * B_LOC], fp32,
                           kind="ExternalOutput")

    with tile.TileContext(nc) as tc:
        with (
            tc.tile_pool(name="const", bufs=1) as cp,
            tc.tile_pool(name="ering", bufs=2) as ep,
            tc.tile_pool(name="e2ring", bufs=2) as ep2,
            tc.tile_pool(name="e3ring", bufs=2) as ep3,
            tc.tile_pool(name="psS", bufs=2, space=bass.MemorySpace.PSUM) as psS,
        ):
            # preload the ACT exp table while input DMAs are in flight
            warm = cp.tile([1, 2], fp32, tag="warm")
            nc.vector.memset(warm[:, :], 0.0)
            nc.scalar.activation(warm[:, 0:1], warm[:, 1:2], AF.Exp)

            xst = cp.tile([44, F * 128], bf16, tag="xst")
            wmv = cp.tile([44, F * 256], bf16, tag="wmv")
            R = cp.tile([128, NCH * B_LOC], fp32, tag="R")

            # input pieces: small first (fast pipeline start), then coarse
            pieces = [1, 1, 1, 1, 2, 2, 4, 4, 4, 4, 4, 4]
            g0 = 0
            for pi, ng in enumerate(pieces):
                nc.sync.dma_start(xst[:, g0 * 512:(g0 + ng) * 512],
                                  d_xst.ap()[:, g0 * 512:(g0 + ng) * 512])
                nc.gpsimd.dma_start(wmv[:, g0 * 1024:(g0 + ng) * 1024],
                                    d_wmv.ap()[:, g0 * 1024:(g0 + ng) * 1024])
                g0 += ng

            for g in range(NGRP):
                sS = psS.tile([128, GCOLS], fp32, tag="S", name="sS")
                for cc in range(CPG):
                    c = g * CPG + cc
                    j, band = c >> 1, c & 1
                    r0 = 32 * band
                    # concurrent row-tile matmuls must not share a PSUM
                    # bank: band 0 -> banks 0-1, band 1 -> banks 2-3
                    pc = (cc & 1) * 1024 + (cc >> 1) * 256
                    nc.tensor.matmul(
                        sS[:, pc:pc + 256],
                        xst[r0:r0 + NK, j * 128:(j + 1) * 128],
                        wmv[r0:r0 + NK, j * 256:(j + 1) * 256],
                        start=True, stop=True,
                        tile_position=(r0, 0),
                    )
                E = ep.tile([128, GCOLS], fp16, tag="E", name="E")
                nc.scalar.activation(E[:, :], sS[:, :], AF.Exp,
                                     scale=float(minus_c))
                # segmented p-sum: two fp16 strided halving adds (2x DVE
                # mode) then a short 16-wide reduce
                Ev = E[:, :].rearrange("p (s x) -> p s x", x=P)
                E2 = ep2.tile([128, GCOLS // 2], fp16, tag="E2", name="E2")
                E2v = E2[:, :].rearrange("p (s x) -> p s x", x=P // 2)
                nc.vector.tensor_tensor(E2v, Ev[:, :, 0:32], Ev[:, :, 32:64],
                                        OP.add)
                E3 = ep3.tile([128, GCOLS // 4], fp16, tag="E3", name="E3")
                E3v = E3[:, :].rearrange("p (s x) -> p s x", x=P // 4)
                nc.vector.tensor_tensor(E3v, E2v[:, :, 0:16], E2v[:, :, 16:32],
                                        OP.add)
                nc.vector.tensor_reduce(
                    R[:, g * 32:(g + 1) * 32], E3v,
                    mybir.AxisListType.X, OP.add)
                if g % 4 == 3:
                    oc = (g // 4) * 128
                    nc.sync.dma_start(d_out.ap()[:, oc:oc + 128],
                                      R[:, oc:oc + 128])

    nc.compile()
    return nc


def _get_nc(minus_c):
    key = float(minus_c)
    if key not in _cache:
        _cache[key] = _build(key)
    return _cache[key]


def _split3(v):
    """3-way bf16 split of an fp64 array: h + m + l ~= v to ~24 bits."""
    h = v.astype(BF16)
    r = v - h.astype(np.float64)
    m = r.astype(BF16)
    r2 = r - m.astype(np.float64)
    l = r2.astype(BF16)
    return h, m, l


def kernel(q2_obs_scaled, amplitude_logits, volumes, filters, sigma,
           _trace=False, _tmpdir=None):
    from concourse.bass_utils import run_bass_kernel_spmd

    sig = float(np.asarray(sigma).reshape(()))
    minus_c = -0.5 / (sig + 0.001) ** 2
    c = -minus_c
    nc = _get_nc(minus_c)

    q = np.asarray(q2_obs_scaled, np.float64)                    # (B, P)
    lg = np.asarray(amplitude_logits, np.float64).reshape(B, F, P)
    vol = np.asarray(volumes, np.float64).reshape(V)
    fil = np.asarray(filters, np.float64).reshape(F)

    # log softmax over p
    mx = lg.max(axis=2, keepdims=True)
    lnamp = lg - (mx + np.log(np.exp(lg - mx).sum(axis=2, keepdims=True)))

    # ---- stationary x-side tile (shared by all cores) ----
    x2d = vol[:, None] * fil[None, :]                            # (V, F)
    xst = np.zeros((44, F * 128), dtype=BF16)
    for band in range(2):
        xT = np.ascontiguousarray(x2d[band * 128:(band + 1) * 128, :].T)
        x2h, x2m, x2l = _split3(xT * xT)
        xh, xm, xl = _split3(xT)
        ones = np.ones_like(xT, dtype=BF16)
        rows = [x2h, x2m, x2l, xh, xh, xh, xm, xm, xl, ones, ones, ones]
        for r, arr in enumerate(rows):
            xst[32 * band + r, :] = np.ascontiguousarray(
                arr, dtype=BF16).reshape(-1)

    # ---- per-core moving q-side tiles ----
    in_maps = []
    for i in range(NCORES):
        qc = q[B_LOC * i:B_LOC * (i + 1), :]                     # (4, P)
        ln = lnamp[B_LOC * i:B_LOC * (i + 1), :, :]              # (4, F, P)
        wh, wm_, wl = _split3(-2.0 * qc)
        ph, pm, pl = _split3(qc[:, None, :] ** 2 - ln / c)       # (4, F, P)

        wmv = np.zeros((44, F * 256), dtype=BF16)
        onesn = np.ones(B_LOC * P, dtype=BF16)

        def flat(a):
            return np.ascontiguousarray(a, dtype=BF16).reshape(-1)

        const_rows = [onesn, onesn, onesn, flat(wh), flat(wm_), flat(wl),
                      flat(wh), flat(wm_), flat(wh)]
        for r, arr in enumerate(const_rows):
            rowv = np.tile(arr, F)
            wmv[r, :] = rowv
            wmv[32 + r, :] = rowv
        for r, arr in enumerate([ph, pm, pl]):
            rowv = flat(arr.transpose(1, 0, 2))                  # (F, 4, P)
            wmv[9 + r, :] = rowv
            wmv[32 + 9 + r, :] = rowv
        in_maps.append({"xst": xst, "wmv": wmv})

    kw = {}
    if _trace:
        kw = {"trace": True, "tmpdir": _tmpdir}
    res = run_bass_kernel_spmd(nc, in_maps, core_ids=list(range(NCORES)), **kw)

    # chunk c = f*2+vh sits at R col base (c//8)*32 + (c%8&1)*16 + (c%8>>1)*4
    cs = np.arange(NCH)
    colbase = (cs // CPG) * 32 + ((cs % CPG) & 1) * 16 + ((cs % CPG) >> 1) * 4
    cols = colbase[:, None] + np.arange(B_LOC)[None, :]          # (NCH, 4)

    out = np.empty((B, V, F), dtype=np.float32)
    for i in range(NCORES):
        Rg = res.results[i]["out"][:, cols]                      # (128, NCH, 4)
        Rr = Rg.reshape(128, F, 2, B_LOC)                        # v',f,vh,b
        out[B_LOC * i:B_LOC * (i + 1)] = (
            Rr.transpose(3, 2, 0, 1).reshape(B_LOC, V, F))
    if _trace:
        return out, res
    return out


# revision 11
# speedup vs baseline: 1.4847x; 1.0379x over previous
"""Trainium2 Bass kernel for nn_ExtractionLayer (v3, windowed sparsity).

metric[b,v,f] = sum_p amp[b,f,p] * exp(-c*(vol[v]*filt[f] - q[b,p])^2)
  amp = softmax_p(logits[b,f,p]),  c = 0.5/(sigma+0.001)^2

Sharding: data-parallel over batch B=32 -> 4 b's per core on 8 cores.

v2 recap (transposed layout): chunks (f, vh) put 128 v's on PSUM
partitions and (b,p) on the free axis; a K=12 bf16 matmul per chunk
computes S = x^2 - 2qx + q^2 - lnamp/c (softmax amp folded into the
exponent), ACT does E = exp(-c*S), DVE does the segmented p-sum.

v3 adds windowed sparsity: exp(-c*d^2) < 1e-6 once |d| > sqrt(14/c)
(~0.15 here), so for each chunk only the q[b,p] inside the chunk's
x-range (+/- thr) can contribute. v is pre-sorted by vol so each
v-half spans ~0.5*filt[f] in x. Per chunk we keep
Ks = ceil8(max_b #selected) p-slots per b (max over the GLOBAL batch
so all 8 SPMD cores share one schedule; dropped terms < 64*e^-14).
Mean Ks ~31 of 64 => ~2x fewer exp/matmul/reduce columns.

Scheduling: chunks sorted by Ks, paired; pair = (band0 chunk, band1
chunk) with equal padded Ks. Groups of pairs fill a [128, 2048] PSUM
tile: band 0 -> cols [0,1024) (banks 0-1), band 1 -> [1024,2048)
(banks 2-3) -- concurrent row-tile matmuls must never share a PSUM
bank. All chunks of a group share one Ks (padded to the group max) so
the group needs one EXP and one fp16 halving add + one 16-ish-wide
reduce. Pad columns carry phi=100 so exp() -> 0 exactly.

ALL small tensors are precomputed on host in fp64 and shipped as two
bf16 input tiles; the schedule is baked per (sigma, selection counts)
and cached.
"""

import sys

for _p in ("/opt/trn_rl_repo", "/root/.axon_site/_ro/trn_rl_repo"):
    if _p not in sys.path:
        sys.path.append(_p)

import numpy as np
import ml_dtypes

BF16 = ml_dtypes.bfloat16

B, V, F, P = 32, 256, 128, 64
NCORES = 8
B_LOC = B // NCORES          # 4 batches per core
NCH = 2 * F                  # 256 chunks: (f, vh)
NK = 12                      # matmul contraction rows
HALF = 1024                  # psum cols per band-half (2 banks)
THR_LN = 14.0                # keep q with c*(x-q)^2 <= THR_LN at window edge
PAD_PHI = 100.0              # phi for padding columns -> exp(-c*100) == 0

_cache: dict = {}


class Schedule:
    """Data-dependent but core-independent processing plan."""

    def __init__(self, Ks_chunk, order):
        # order: list of chunk ids sorted by Ks desc; pairs = (order[2i],
        # order[2i+1]) with pair Ks = max of the two; groups pack pairs
        # with a shared (max) Ks such that npairs*4*Ks <= HALF.
        self.pairs = []                  # (chunkA, chunkB, Ks_pair)
        for i in range(0, NCH, 2):
            a, b = order[i], order[i + 1]
            self.pairs.append((a, b, max(Ks_chunk[a], Ks_chunk[b])))
        self.groups = []                 # list of (pair_lo, npairs, Ks_grp)
        i = 0
        while i < len(self.pairs):
            Kg = self.pairs[i][2]
            n = 1
            while (i + n < len(self.pairs)
                   and (n + 1) * 4 * Kg <= HALF):
                n += 1
            self.groups.append((i, n, Kg))
            i += n
        # flat emission order of chunks: per group, per pair: A then B
        self.chunks = []                 # (chunk_id, band, group, slot)
        for gi, (plo, npair, Kg) in enumerate(self.groups):
            for s in range(npair):
                a, b, _ = self.pairs[plo + s]
                self.chunks.append((a, 0, gi, s))
                self.chunks.append((b, 1, gi, s))
        # R column map: chunk -> base col of its 4 b sums.
        # reduce out for group gi half H: cols rbase(g) + H*4*npair + s*4 + b
        self.rbase = []
        acc = 0
        for (plo, npair, Kg) in self.groups:
            self.rbase.append(acc)
            acc += 8 * npair
        self.rtot = acc                  # == 8 * 128 == 1024
        self.rcol = {}                   # chunk_id -> base col
        for (cid, band, gi, s) in self.chunks:
            plo, npair, Kg = self.groups[gi]
            self.rcol[cid] = self.rbase[gi] + band * 4 * npair + s * 4
        # wmv col offset per chunk (emission order, 4*Ks_grp cols each)
        self.woff = {}
        acc = 0
        for (cid, band, gi, s) in self.chunks:
            Kg = self.groups[gi][2]
            self.woff[cid] = acc
            acc += 4 * Kg
        self.wtot = acc
        # xst col offset: 128 per chunk in emission order
        self.xoff = {cid: k * 128 for k, (cid, _, _, _) in
                     enumerate(self.chunks)}
        self.key = (tuple(Ks_chunk), tuple(order))


def _build(minus_c, sched):
    import concourse.tile as tile
    from concourse import bacc, mybir

    fp32 = mybir.dt.float32
    fp16 = mybir.dt.float16
    bf16 = mybir.dt.bfloat16
    AF = mybir.ActivationFunctionType
    OP = mybir.AluOpType
    import concourse.bass as bass

    nc = bacc.Bacc("TRN2", target_bir_lowering=False, debug=False,
                   num_devices=NCORES)

    d_xst = nc.dram_tensor("xst", [44, NCH * 128], bf16,
                           kind="ExternalInput")
    d_wmv = nc.dram_tensor("wmv", [44, sched.wtot], bf16,
                           kind="ExternalInput")
    d_out = nc.dram_tensor("out", [128, sched.rtot], fp32,
                           kind="ExternalOutput")

    ngroups = len(sched.groups)
    # chunk list per group for emission
    by_group = [[] for _ in range(ngroups)]
    for (cid, band, gi, s) in sched.chunks:
        by_group[gi].append((cid, band, s))

    with tile.TileContext(nc) as tc:
        with (
            tc.tile_pool(name="const", bufs=1) as cp,
            tc.tile_pool(name="ering", bufs=2) as ep,
            tc.tile_pool(name="e2ring", bufs=2) as ep2,
            tc.tile_pool(name="psS", bufs=2, space=bass.MemorySpace.PSUM) as psS,
        ):
            warm = cp.tile([1, 2], fp32, tag="warm")
            nc.vector.memset(warm[:, :], 0.0)
            nc.scalar.activation(warm[:, 0:1], warm[:, 1:2], AF.Exp)

            xst = cp.tile([44, NCH * 128], bf16, tag="xst")
            wmv = cp.tile([44, sched.wtot], bf16, tag="wmv")
            R = cp.tile([128, sched.rtot], fp32, tag="R")

            # input pieces along group boundaries: fine first, then coarse
            gsz = [1, 1, 1, 1, 2, 2]
            while sum(gsz) < ngroups:
                gsz.append(min(4, ngroups - sum(gsz)))
            g0 = 0
            for ng in gsz:
                glast = min(g0 + ng, ngroups) - 1
                ca = by_group[g0][0][0]
                cz = by_group[glast][-1][0]
                x0, x1 = sched.xoff[ca], sched.xoff[cz] + 128
                plo, npair, Kg = sched.groups[glast]
                w0 = sched.woff[ca]
                w1 = sched.woff[cz] + 4 * Kg
                nc.sync.dma_start(xst[:, x0:x1], d_xst.ap()[:, x0:x1])
                nc.gpsimd.dma_start(wmv[:, w0:w1], d_wmv.ap()[:, w0:w1])
                g0 += ng

            ocursor = 0
            for gi in range(ngroups):
                plo, npair, Kg = sched.groups[gi]
                h = npair * 4 * Kg       # cols per half
                sS = psS.tile([128, 2 * HALF], fp32, tag="S", name="sS")
                for (cid, band, s) in by_group[gi]:
                    r0 = 32 * band
                    xo = sched.xoff[cid]
                    wo = sched.woff[cid]
                    pc = band * HALF + s * 4 * Kg
                    nc.tensor.matmul(
                        sS[:, pc:pc + 4 * Kg],
                        xst[r0:r0 + NK, xo:xo + 128],
                        wmv[r0:r0 + NK, wo:wo + 4 * Kg],
                        start=True, stop=True,
                        tile_position=(r0, 0),
                    )
                E = ep.tile([128, 2 * HALF], fp16, tag="E", name="E")
                Sv = sS[:, :].rearrange("p (u x) -> p u x", u=2)[:, :, 0:h]
                Ev = E[:, :].rearrange("p (u x) -> p u x", u=2)[:, :, 0:h]
                nc.scalar.activation(Ev, Sv, AF.Exp, scale=float(minus_c))
                # p-sum: one fp16 halving add (2x mode) + one Kg/2 reduce
                nseg = npair * 4
                E4 = (E[:, :].rearrange("p (u y) -> p u y", u=2)
                      [:, :, 0:nseg * Kg]
                      .rearrange("p u (s x) -> p u s x", x=Kg))
                E2 = ep2.tile([128, HALF], fp16, tag="E2", name="E2")
                E2v = (E2[:, :].rearrange("p (u y) -> p u y", u=2)
                       [:, :, 0:nseg * (Kg // 2)]
                       .rearrange("p u (s x) -> p u s x", x=Kg // 2))
                nc.vector.tensor_tensor(E2v, E4[:, :, :, 0:Kg // 2],
                                        E4[:, :, :, Kg // 2:Kg], OP.add)
                nc.vector.tensor_reduce(
                    R[:, sched.rbase[gi]:sched.rbase[gi] + 8 * npair]
                    .rearrange("p (u s) -> p u s", u=2),
                    E2v, mybir.AxisListType.X, OP.add)
                # stream out finished R spans every few groups
                rend = sched.rbase[gi] + 8 * npair
                if gi % 3 == 2 or gi == ngroups - 1:
                    nc.sync.dma_start(d_out.ap()[:, ocursor:rend],
                                      R[:, ocursor:rend])
                    ocursor = rend

    nc.compile()
    return nc


def _get_nc(minus_c, sched):
    key = (float(minus_c), sched.key)
    if key not in _cache:
        _cache[key] = _build(minus_c, sched)
    return _cache[key]


def _split3(v):
    """3-way bf16 split of an fp64 array: h + m + l ~= v to ~24 bits."""
    h = v.astype(BF16)
    r = v - h.astype(np.float64)
    m = r.astype(BF16)
    r2 = r - m.astype(np.float64)
    l = r2.astype(BF16)
    return h, m, l


def kernel(q2_obs_scaled, amplitude_logits, volumes, filters, sigma,
           _trace=False, _tmpdir=None):
    from concourse.bass_utils import run_bass_kernel_spmd

    sig = float(np.asarray(sigma).reshape(()))
    minus_c = -0.5 / (sig + 0.001) ** 2
    c = -minus_c
    thr = np.sqrt(THR_LN / c)

    q = np.asarray(q2_obs_scaled, np.float64)                    # (B, P)
    lg = np.asarray(amplitude_logits, np.float64).reshape(B, F, P)
    vol = np.asarray(volumes, np.float64).reshape(V)
    fil = np.asarray(filters, np.float64).reshape(F)

    mx = lg.max(axis=2, keepdims=True)
    lnamp = lg - (mx + np.log(np.exp(lg - mx).sum(axis=2, keepdims=True)))

    # ---- schedule: windowed selection, global over the batch ----
    vperm = np.argsort(vol, kind="stable")
    vs = vol[vperm]
    xs = vs[:, None] * fil[None, :]                              # (V, F)
    sel = [None] * NCH                                           # (B, P) bool
    Ks_chunk = [0] * NCH
    for cid in range(NCH):
        f, vh = cid >> 1, cid & 1
        xw = xs[vh * 128:(vh + 1) * 128, f]
        lo, hi = xw.min() - thr, xw.max() + thr
        m = (q >= lo) & (q <= hi)                                # (B, P)
        sel[cid] = m
        # power-of-two K so same-size PSUM slots never straddle a bank
        n = int(m.sum(axis=1).max())
        Ks_chunk[cid] = next(k for k in (8, 16, 32, 64) if k >= n)
    order = sorted(range(NCH), key=lambda cix: -Ks_chunk[cix])
    sched = Schedule(Ks_chunk, order)
    nc = _get_nc(minus_c, sched)

    # ---- stationary x-side tile (shared by all cores) ----
    xst = np.zeros((44, NCH * 128), dtype=BF16)
    for (cid, band, gi, s) in sched.chunks:
        f, vh = cid >> 1, cid & 1
        xw = xs[vh * 128:(vh + 1) * 128, f]                      # (128,)
        x2h, x2m, x2l = _split3(xw * xw)
        xh, xm, xl = _split3(xw)
        ones = np.ones(128, dtype=BF16)
        rows = [x2h, x2m, x2l, xh, xh, xh, xm, xm, xl, ones, ones, ones]
        xo = sched.xoff[cid]
        for r, arr in enumerate(rows):
            xst[32 * band + r, xo:xo + 128] = arr

    # ---- per-core moving q-side tiles ----
    wh_a, wm_a, wl_a = _split3(-2.0 * q)                         # (B, P)
    phi = q[:, None, :] ** 2 - lnamp / c                         # (B, F, P)
    ph_a, pm_a, pl_a = _split3(phi)

    in_maps = []
    for i in range(NCORES):
        wmv = np.zeros((44, sched.wtot), dtype=BF16)
        for (cid, band, gi, s) in sched.chunks:
            f = cid >> 1
            Kg = sched.groups[gi][2]
            wo = sched.woff[cid]
            r0 = 32 * band
            for bl in range(B_LOC):
                bg = B_LOC * i + bl
                ps = np.nonzero(sel[cid][bg])[0]
                n = len(ps)
                col = wo + bl * Kg
                wmv[r0 + 0, col:col + Kg] = 1.0
                wmv[r0 + 1, col:col + Kg] = 1.0
                wmv[r0 + 2, col:col + Kg] = 1.0
                wmv[r0 + 3, col:col + n] = wh_a[bg, ps]
                wmv[r0 + 4, col:col + n] = wm_a[bg, ps]
                wmv[r0 + 5, col:col + n] = wl_a[bg, ps]
                wmv[r0 + 6, col:col + n] = wh_a[bg, ps]
                wmv[r0 + 7, col:col + n] = wm_a[bg, ps]
                wmv[r0 + 8, col:col + n] = wh_a[bg, ps]
                wmv[r0 + 9, col:col + n] = ph_a[bg, f, ps]
                wmv[r0 + 10, col:col + n] = pm_a[bg, f, ps]
                wmv[r0 + 11, col:col + n] = pl_a[bg, f, ps]
                if n < Kg:
                    wmv[r0 + 9, col + n:col + Kg] = PAD_PHI
        in_maps.append({"xst": xst, "wmv": wmv})

    kw = {}
    if _trace:
        kw = {"trace": True, "tmpdir": _tmpdir}
    res = run_bass_kernel_spmd(nc, in_maps, core_ids=list(range(NCORES)), **kw)

    # ---- host unpack: R[v'(sorted), rcol[cid]+b] -> out[b, v, f] ----
    vback = vperm.reshape(2, 128)                                # vh, v'
    out = np.empty((B, V, F), dtype=np.float32)
    rc = np.array([sched.rcol[cid] for cid in range(NCH)])       # (NCH,)
    for i in range(NCORES):
        R = res.results[i]["out"]                                # (128, rtot)
        cols = rc[:, None] + np.arange(B_LOC)[None, :]           # (NCH, 4)
        Rg = R[:, cols]                                          # (128,NCH,4)
        for bl in range(B_LOC):
            o = out[B_LOC * i + bl]                              # (V, F)
            g = Rg[:, :, bl].reshape(128, F, 2)                  # v', f, vh
            for vh in range(2):
                o[vback[vh], :] = g[:, :, vh]
    if _trace:
        return out, res
    return out
